# revision 65
# baseline (speedup 1.0000x reference)
import sys

sys.path.insert(0, "/opt/trn_rl_repo")

import numpy as np

# ---- problem constants (hardcoded from the nn_LocalAggregator spec) ----
PC_MIN = np.array([-40.0, -40.0, -1.0], dtype=np.float32)
GRID = np.float32(0.4)
SCALE_MULT = np.float32(3.0)
N_PTS, N_GAUSS, N_CLS = 16384, 4096, 18
N_CORES = 8
NPC = N_PTS // N_CORES          # 2048 points per core
BLK = 256                       # point block (matmul free dim)
NBLK = NPC // BLK               # 8
P = 128                         # partitions / gaussians per tile
KQ = 9                          # quadratic+linear monomial features
NSPLIT = [(0, 0), (0, 1), (1, 0)]  # fp16 2-level split combos
KQR = KQ * len(NSPLIT)          # 27 quad rows after splitting
NBIAS = 2                       # bias folded in as 2 fp16 rows (feat = 1)
KQB = KQR + NBIAS               # 29 dense rows
KY_WIN = 17                     # in-window y voxels
KYR = KY_WIN + 1                # + 1 "stray" row (no y penalty, addend fixes)
KZ = 16                         # z voxel range
GRP = 4                         # pairs per merged exp (one 2-bank psum tile)

FP16 = np.float16
MASKVAL = np.float64(240.0)     # mask penalty per violated axis (fp16-exact)

# module global for test harness introspection (exec time etc.)
LAST_RESULTS = None


def _split(x, n):
    """float64 array -> n fp16 levels whose sum approximates x."""
    out = []
    r = np.asarray(x, dtype=np.float64)
    for _ in range(n):
        a = r.astype(FP16)
        out.append(a)
        r = r - a.astype(np.float64)
    return out


def _prep(pts, means3D, opacities, semantics, scales, cov3D):
    """Host-side prep: sharding, features, coefficient tables."""
    p = np.asarray(pts[0], dtype=np.float32)          # [N,3]
    mu = np.asarray(means3D[0], dtype=np.float32)     # [M,3]
    opa = np.asarray(opacities[0], dtype=np.float32)  # [M]
    sem = np.asarray(semantics[0], dtype=np.float32)  # [M,C]
    sc = np.asarray(scales[0], dtype=np.float32)      # [M,3]
    cov = np.asarray(cov3D[0], dtype=np.float32)      # [M,3,3]

    # integer voxel coords / radii -- fp32 ops exactly as the reference
    p_int = ((p - PC_MIN) / GRID).astype(np.int32)
    m_int = ((mu - PC_MIN) / GRID).astype(np.int32)
    radii = np.ceil(sc.max(axis=-1) * SCALE_MULT / GRID).astype(np.int32)

    cxx = cov[:, 0, 0].astype(np.float64)
    cyy = cov[:, 1, 1].astype(np.float64)
    czz = cov[:, 2, 2].astype(np.float64)
    cxy = cov[:, 0, 1].astype(np.float64)
    cyz = cov[:, 1, 2].astype(np.float64)
    cxz = cov[:, 0, 2].astype(np.float64)
    with np.errstate(divide="ignore"):
        lnopa = np.maximum(np.log(opa.astype(np.float64)), -20000.0)

    # ---- shard points: equal x-chunks, y-sorted inside each core; pick the
    # sort direction so the wide-span (stray) block is always the LAST one ----
    order_x = np.argsort(p[:, 0], kind="stable")
    core_idx = []
    for c in range(N_CORES):
        idx = order_x[c * NPC:(c + 1) * NPC]
        idx = idx[np.argsort(p[idx, 1], kind="stable")]
        vy = p_int[idx, 1]
        if vy[:BLK].max() - vy[:BLK].min() > vy[-BLK:].max() - vy[-BLK:].min():
            idx = idx[::-1]
        core_idx.append(idx)

    # ---- per-core gaussian subsets (x-reach cull), y-sorted ----
    core_gsel = []
    kx = 0
    for c in range(N_CORES):
        vx = p_int[core_idx[c], 0]
        kx = max(kx, int(vx.max() - vx.min()) + 1)
        m = (m_int[:, 0] >= vx.min() - radii) & (m_int[:, 0] <= vx.max() + radii)
        gsel = np.nonzero(m)[0]
        gsel = gsel[np.argsort(m_int[gsel, 1], kind="stable")]
        core_gsel.append(gsel)

    KT = KQB + kx + KYR + KZ            # total contraction rows
    assert KT <= 128, KT
    R_X = KQB
    R_Y = KQB + kx
    R_Z = KQB + kx + KYR

    # ---- per-core block windows + pair lists; pad across cores (SPMD) ----
    # each block gets PRIVATE tiles: its in-reach gaussians packed densely
    # into ceil(|S|/128) tiles (dup across blocks is fine; within a block
    # every in-reach gaussian appears exactly once)
    def _block_pass(c, idx, check_suffix):
        vy = p_int[idx, 1]
        gs = core_gsel[c]
        wlos, tb, strays = [], [], []
        for b in range(NBLK):
            vyb = vy[b * BLK:(b + 1) * BLK]
            desc = vyb[0] > vyb[-1]
            wlo = int(vyb[0]) - (KY_WIN - 1) if desc else int(vyb[0])
            wlos.append(wlo)
            inw = (vyb >= wlo) & (vyb < wlo + KY_WIN)
            ns = int((~inw).sum())
            strays.append(ns)
            if ns and check_suffix:
                assert inw[:BLK - ns].all(), "strays must be a suffix"
                assert b == NBLK - 1, "strays only in last slot"
            ylo, yhi = int(vyb.min()), int(vyb.max())
            S = gs[(m_int[gs, 1] + radii[gs] >= ylo)
                   & (m_int[gs, 1] - radii[gs] <= yhi)]
            lst = [S[k * P:(k + 1) * P]
                   for k in range(int(np.ceil(len(S) / P)))]
            tb.append(lst)
        return wlos, tb, strays

    # assign each core's blocks to SPMD slots so the padded per-slot pair
    # counts (elementwise max over cores) are minimal: stray block pinned to
    # the last slot, the rest sorted by pair count descending
    core_info = []
    npb = np.zeros(NBLK, dtype=np.int64)
    smax = 0
    for c in range(N_CORES):
        _, tb0, strays0 = _block_pass(c, core_idx[c], False)
        stray_bs = [b for b in range(NBLK) if strays0[b] > 0]
        assert len(stray_bs) <= 1
        sb = stray_bs[0] if stray_bs else \
            int(np.argmin([len(l) for l in tb0]))
        rest = sorted((b for b in range(NBLK) if b != sb),
                      key=lambda b: -len(tb0[b]))
        perm = rest + [sb]
        idx = np.concatenate([core_idx[c][b * BLK:(b + 1) * BLK]
                              for b in perm])
        core_idx[c] = idx
        wlos, tb, strays = _block_pass(c, idx, True)
        smax = max(smax, max(strays))
        for b in range(NBLK):
            npb[b] = max(npb[b], len(tb[b]))
        core_info.append((wlos, tb, strays))
    S = max(int(smax), 16)
    npair = int(npb.sum())
    # process the stray/addend slot (7) early, where its vector-engine mask
    # adds overlap pipeline slack; end on cheap 2-pair slots
    border = [0, 1, 7, 2, 3, 4, 5, 6]
    pair_block = []
    for b in border:
        pair_block += [b] * int(npb[b])
    add_pairs = [i for i, b in enumerate(pair_block) if b == NBLK - 1]

    # ---- per-core device arrays ----
    in_maps = []
    for c in range(N_CORES):
        idx = core_idx[c]
        gsel = core_gsel[c]
        wlos, tb, strays = core_info[c]
        vx = p_int[idx, 0]
        vy = p_int[idx, 1]
        vz = p_int[idx, 2]
        vx_lo = int(vx.min())
        pc = p[idx].astype(np.float64)

        feat = np.zeros((KT, NPC), dtype=FP16)
        stat = np.zeros((KT, npair * P), dtype=FP16)
        semt = np.zeros((P, npair * N_CLS), dtype=FP16)
        addn = np.zeros((P, len(add_pairs) * S), dtype=FP16)

        centers = np.stack([pc[b * BLK:(b + 1) * BLK].mean(axis=0)
                            for b in range(NBLK)])

        for b in range(NBLK):
            cols = slice(b * BLK, (b + 1) * BLK)
            dd = pc[cols] - centers[b]
            x, y, z = dd[:, 0], dd[:, 1], dd[:, 2]
            q = np.stack([x * x, y * y, z * z, x * y, y * z, x * z, x, y, z])
            qs = _split(q, 2)
            for f in range(KQ):
                for k, (i, _) in enumerate(NSPLIT):
                    feat[f * len(NSPLIT) + k, cols] = qs[i][f]
            feat[KQR:KQR + NBIAS, cols] = FP16(1)
            ar = np.arange(b * BLK, (b + 1) * BLK)
            feat[R_X + (vx[cols] - vx_lo), ar] = FP16(1)
            yr = vy[cols] - wlos[b]
            yrow = np.where((yr < 0) | (yr >= KY_WIN), KY_WIN,
                            np.clip(yr, 0, KY_WIN))
            feat[R_Y + yrow, ar] = FP16(1)
            feat[R_Z + vz[cols], ar] = FP16(1)

        pi = 0
        for b in border:
            lst = tb[b]
            for j in range(int(npb[b])):
                if j < len(lst):
                    gg = lst[j]
                    ng = len(gg)
                    mup = mu[gg].astype(np.float64) - centers[b]
                    mx, my, mz = mup[:, 0], mup[:, 1], mup[:, 2]
                    gxx, gyy, gzz = cxx[gg], cyy[gg], czz[gg]
                    gxy, gyz, gxz = cxy[gg], cyz[gg], cxz[gg]
                    hx = gxx * mx + gxy * my + gxz * mz
                    hy = gxy * mx + gyy * my + gyz * mz
                    hz = gxz * mx + gyz * my + gzz * mz
                    gq = np.stack([-0.5 * gxx, -0.5 * gyy, -0.5 * gzz,
                                   -gxy, -gyz, -gxz, hx, hy, hz])
                    gsp = _split(gq, 2)
                    sl = slice(pi * P, pi * P + ng)
                    for f in range(KQ):
                        for k, (_, jj) in enumerate(NSPLIT):
                            stat[f * len(NSPLIT) + k, sl] = gsp[jj][f]
                    quad = (gxx * mx * mx + gyy * my * my + gzz * mz * mz
                            + 2 * gxy * mx * my + 2 * gyz * my * mz
                            + 2 * gxz * mx * mz)
                    bias = -0.5 * quad + lnopa[gg]
                    for k, bsp in enumerate(_split(bias, NBIAS)):
                        stat[KQR + k, sl] = bsp
                    vv = np.arange(kx)[:, None] + vx_lo
                    out_x = np.abs(vv - m_int[gg, 0][None, :]) > radii[gg][None, :]
                    stat[R_X:R_X + kx, sl] = np.where(out_x, -MASKVAL, 0.0).astype(FP16)
                    vv = np.arange(KY_WIN)[:, None] + wlos[b]
                    out_y = np.abs(vv - m_int[gg, 1][None, :]) > radii[gg][None, :]
                    stat[R_Y:R_Y + KY_WIN, sl] = np.where(out_y, -MASKVAL, 0.0).astype(FP16)
                    # stray row R_Y + KY_WIN stays 0 (mask applied via addend)
                    vv = np.arange(KZ)[:, None]
                    out_z = np.abs(vv - m_int[gg, 2][None, :]) > radii[gg][None, :]
                    stat[R_Z:R_Z + KZ, sl] = np.where(out_z, -MASKVAL, 0.0).astype(FP16)
                    semt[:ng, pi * N_CLS:(pi + 1) * N_CLS] = sem[gg]
                    # addend: exact y mask for the stray suffix of last block
                    if b == NBLK - 1 and strays[b] > 0:
                        k = add_pairs.index(pi)
                        ns = strays[b]
                        svy = vy[b * BLK + BLK - ns:(b + 1) * BLK]
                        out_sy = (np.abs(m_int[gg, 1][:, None] - svy[None, :])
                                  > radii[gg][:, None])
                        addn[:ng, k * S + S - ns:(k + 1) * S] = \
                            np.where(out_sy, -MASKVAL, 0.0).astype(FP16)
                pi += 1

        in_maps.append({"feat": feat, "stat": stat, "semt": semt, "addn": addn})

    meta = dict(npair=npair, pair_block=pair_block, KT=KT,
                core_idx=core_idx, npb=npb, S=S, add_pairs=add_pairs)
    return in_maps, meta


def _build_nc(npair, pair_block, KT, S, add_pairs):
    import concourse.bass as bass  # noqa: F401
    import concourse.mybir as mybir
    import concourse.tile as tile
    from concourse import bacc

    f32 = mybir.dt.float32
    fp16 = mybir.dt.float16

    nc = bacc.Bacc("TRN2", target_bir_lowering=False, debug=False,
                   num_devices=N_CORES)
    feat_d = nc.dram_tensor("feat", [KT, NPC], fp16, kind="ExternalInput")
    stat_d = nc.dram_tensor("stat", [KT, npair * P], fp16, kind="ExternalInput")
    semt_d = nc.dram_tensor("semt", [P, npair * N_CLS], fp16, kind="ExternalInput")
    addn_d = nc.dram_tensor("addn", [P, len(add_pairs) * S], fp16,
                            kind="ExternalInput")
    out_d = nc.dram_tensor("out", [N_CLS, NPC], f32, kind="ExternalOutput")

    first = {}
    last = {}
    for i, b in enumerate(pair_block):
        first.setdefault(b, i)
        last[b] = i
    # groups of GRP pairs; shave one pair off the head group (releases the
    # first psum buffer sooner without adding a group) and keep a short tail
    grp_pairs = []
    i = 0
    while i < npair:
        n = GRP - 1 if i == 0 else (GRP if npair - i > 4 else 2)
        grp_pairs.append(list(range(i, min(i + n, npair))))
        i += n
    ngrp = len(grp_pairs)

    KH = KT // 2

    with tile.TileContext(nc) as tc:
        with (
            tc.tile_pool(name="resident", bufs=1) as res_pool,
            tc.tile_pool(name="wpool", bufs=4) as w_pool,
            tc.tile_pool(name="pwp", bufs=2, space="PSUM") as pw_pool,
            tc.tile_pool(name="lgp", bufs=1, space="PSUM") as lg_pool,
        ):
            feat_s = res_pool.tile([KT, NPC], fp16, name="feat_s")
            stat_s = res_pool.tile([KT, npair * P], fp16, name="stat_s")
            semt_s = res_pool.tile([P, npair * N_CLS], fp16, name="semt_s")
            addn_s = res_pool.tile([P, len(add_pairs) * S], fp16, name="addn_s")
            out_s = res_pool.tile([N_CLS, NPC], f32, name="out_s")

            def fdma(eng, rows, cols):
                eng.dma_start(out=feat_s[rows, cols], in_=feat_d[rows, cols])

            def sdma(eng, rows, plo, phi):
                eng.dma_start(out=stat_s[rows, plo * P:phi * P],
                              in_=stat_d[rows, plo * P:phi * P])

            k3 = KT // 3
            r1, r2, r3 = slice(0, k3), slice(k3, 2 * k3), slice(2 * k3, KT)
            allc = slice(0, NPC)

            # row-thirds across all three queues; small column-head feat
            # chunks (group-0 blocks) let the first matmuls start early,
            # then full-width rows (4KB bursts) for the rest
            rall = slice(0, KT)
            hd = slice(0, 2 * BLK)
            rst = slice(2 * BLK, NPC)
            c1, c2, c3, c4 = (npair * k // 5 for k in (1, 2, 3, 4))
            fdma(nc.sync, r1, hd)
            sdma(nc.sync, rall, 0, c1)
            fdma(nc.sync, r1, rst)
            sdma(nc.sync, rall, c3, c4)

            fdma(nc.scalar, r2, hd)
            fdma(nc.scalar, r2, rst)
            nc.scalar.dma_start(out=semt_s[:], in_=semt_d[:])
            sdma(nc.scalar, rall, c4, npair)

            fdma(nc.gpsimd, r3, hd)
            sdma(nc.gpsimd, rall, c1, c2)
            fdma(nc.gpsimd, r3, rst)
            nc.gpsimd.dma_start(out=addn_s[:], in_=addn_d[:])
            sdma(nc.gpsimd, rall, c2, c3)

            lg = [lg_pool.tile([N_CLS, 2 * BLK], f32, name=f"lg{k}")
                  for k in range(NBLK // 2)]

            pw = [None] * ngrp
            w = [None] * ngrp

            def emit_splats(g):
                pw[g] = pw_pool.tile([P, len(grp_pairs[g]) * BLK], f32,
                                     name="pw")
                for s, i in enumerate(grp_pairs[g]):
                    b = pair_block[i]
                    nc.tensor.matmul(
                        out=pw[g][:, s * BLK:(s + 1) * BLK],
                        lhsT=stat_s[:, i * P:(i + 1) * P],
                        rhs=feat_s[:, b * BLK:(b + 1) * BLK],
                        start=True, stop=True)
                    if i in add_pairs:
                        k = add_pairs.index(i)
                        nc.vector.scalar_tensor_tensor(
                            out=pw[g][:, (s + 1) * BLK - S:(s + 1) * BLK],
                            in0=addn_s[:, k * S:(k + 1) * S],
                            scalar=1.0,
                            in1=pw[g][:, (s + 1) * BLK - S:(s + 1) * BLK],
                            op0=mybir.AluOpType.mult,
                            op1=mybir.AluOpType.add)

            def emit_exp(g):
                n = len(grp_pairs[g]) * BLK
                w[g] = w_pool.tile([P, n], fp16, name="w")
                nc.scalar.activation(w[g][:], pw[g][:],
                                     mybir.ActivationFunctionType.Exp)

            def emit_semts(g):
                for s, i in enumerate(grp_pairs[g]):
                    b = pair_block[i]
                    k, h = b // 2, b % 2
                    nc.tensor.matmul(
                        out=lg[k][:, h * BLK:(h + 1) * BLK],
                        lhsT=semt_s[:, i * N_CLS:(i + 1) * N_CLS],
                        rhs=w[g][:, s * BLK:(s + 1) * BLK],
                        start=(first[b] == i), stop=(last[b] == i))
                    if k == NBLK // 2 - 1 and last[b] == i:
                        # last bank: slot 7 completes mid-stream, so copy
                        # each half as its block finishes — only a 256-col
                        # copy remains on the critical tail
                        hs = slice((2 * k + h) * BLK, (2 * k + h + 1) * BLK)
                        nc.vector.tensor_copy(out_s[:, hs],
                                              lg[k][:, h * BLK:(h + 1) * BLK])
                    if max(last[2 * k], last[2 * k + 1]) == i:
                        sl = slice(k * 2 * BLK, (k + 1) * 2 * BLK)
                        if k < NBLK // 2 - 1:
                            nc.vector.tensor_copy(out_s[:, sl], lg[k][:])
                        dsl = slice(0, 4 * BLK) if k == 1 else sl
                        if k >= 1:
                            nc.sync.dma_start(out=out_d[:, dsl],
                                              in_=out_s[:, dsl])

            # semts trail the exps by two groups so they never wait on the
            # scalar engine; flush the last two groups at the end
            for g in range(ngrp):
                emit_splats(g)
                emit_exp(g)
                if g >= 2:
                    emit_semts(g - 2)
            emit_semts(ngrp - 2)
            emit_semts(ngrp - 1)

    nc.compile()
    return nc


def kernel(pts, means3D, opacities, semantics, scales, cov3D):
    global LAST_RESULTS
    from concourse.bass_utils import run_bass_kernel_spmd

    in_maps, meta = _prep(pts, means3D, opacities, semantics, scales, cov3D)
    nc = _build_nc(meta["npair"], meta["pair_block"], meta["KT"],
                   meta["S"], meta["add_pairs"])
    res = run_bass_kernel_spmd(nc, in_maps, core_ids=list(range(N_CORES)))
    LAST_RESULTS = res

    out = np.empty((N_PTS, N_CLS), dtype=np.float32)
    for c in range(N_CORES):
        out[meta["core_idx"][c]] = res.results[c]["out"].T
    return out


# revision 66
# speedup vs baseline: 1.0202x; 1.0202x over previous
import sys

sys.path.insert(0, "/opt/trn_rl_repo")

import numpy as np

# ---- problem constants (hardcoded from the nn_LocalAggregator spec) ----
PC_MIN = np.array([-40.0, -40.0, -1.0], dtype=np.float32)
GRID = np.float32(0.4)
SCALE_MULT = np.float32(3.0)
N_PTS, N_GAUSS, N_CLS = 16384, 4096, 18
N_CORES = 8
NPC = N_PTS // N_CORES          # 2048 points per core
BLK = 256                       # point block (matmul free dim)
NBLK = NPC // BLK               # 8
P = 128                         # partitions / gaussians per tile
KQ = 9                          # quadratic+linear monomial features
NSPLIT = [(0, 0), (0, 1), (1, 0)]  # fp16 2-level split combos
KQR = KQ * len(NSPLIT)          # 27 quad rows after splitting
NBIAS = 2                       # bias folded in as 2 fp16 rows (feat = 1)
KQB = KQR + NBIAS               # 29 dense rows
KY_WIN = 17                     # in-window y voxels
KYR = KY_WIN + 1                # + 1 "stray" row (no y penalty, addend fixes)
KZ = 16                         # z voxel range
GRP = 4                         # pairs per merged exp (one 2-bank psum tile)

FP16 = np.float16
MASKVAL = np.float64(240.0)     # mask penalty per violated axis (fp16-exact)

# module global for test harness introspection (exec time etc.)
LAST_RESULTS = None


def _split(x, n):
    """float64 array -> n fp16 levels whose sum approximates x."""
    out = []
    r = np.asarray(x, dtype=np.float64)
    for _ in range(n):
        a = r.astype(FP16)
        out.append(a)
        r = r - a.astype(np.float64)
    return out


def _prep(pts, means3D, opacities, semantics, scales, cov3D):
    """Host-side prep: sharding, features, coefficient tables."""
    p = np.asarray(pts[0], dtype=np.float32)          # [N,3]
    mu = np.asarray(means3D[0], dtype=np.float32)     # [M,3]
    opa = np.asarray(opacities[0], dtype=np.float32)  # [M]
    sem = np.asarray(semantics[0], dtype=np.float32)  # [M,C]
    sc = np.asarray(scales[0], dtype=np.float32)      # [M,3]
    cov = np.asarray(cov3D[0], dtype=np.float32)      # [M,3,3]

    # integer voxel coords / radii -- fp32 ops exactly as the reference
    p_int = ((p - PC_MIN) / GRID).astype(np.int32)
    m_int = ((mu - PC_MIN) / GRID).astype(np.int32)
    radii = np.ceil(sc.max(axis=-1) * SCALE_MULT / GRID).astype(np.int32)

    cxx = cov[:, 0, 0].astype(np.float64)
    cyy = cov[:, 1, 1].astype(np.float64)
    czz = cov[:, 2, 2].astype(np.float64)
    cxy = cov[:, 0, 1].astype(np.float64)
    cyz = cov[:, 1, 2].astype(np.float64)
    cxz = cov[:, 0, 2].astype(np.float64)
    with np.errstate(divide="ignore"):
        lnopa = np.maximum(np.log(opa.astype(np.float64)), -20000.0)

    # ---- shard points: equal x-chunks, y-sorted inside each core; pick the
    # sort direction so the wide-span (stray) block is always the LAST one ----
    order_x = np.argsort(p[:, 0], kind="stable")
    core_idx = []
    for c in range(N_CORES):
        idx = order_x[c * NPC:(c + 1) * NPC]
        idx = idx[np.argsort(p[idx, 1], kind="stable")]
        vy = p_int[idx, 1]
        if vy[:BLK].max() - vy[:BLK].min() > vy[-BLK:].max() - vy[-BLK:].min():
            idx = idx[::-1]
        core_idx.append(idx)

    # ---- per-core gaussian subsets (x-reach cull), y-sorted ----
    core_gsel = []
    kx = 0
    for c in range(N_CORES):
        vx = p_int[core_idx[c], 0]
        kx = max(kx, int(vx.max() - vx.min()) + 1)
        m = (m_int[:, 0] >= vx.min() - radii) & (m_int[:, 0] <= vx.max() + radii)
        gsel = np.nonzero(m)[0]
        gsel = gsel[np.argsort(m_int[gsel, 1], kind="stable")]
        core_gsel.append(gsel)

    KT = KQB + kx + KYR + KZ            # total contraction rows
    assert KT <= 128, KT
    R_X = KQB
    R_Y = KQB + kx
    R_Z = KQB + kx + KYR

    # ---- per-core block windows + pair lists; pad across cores (SPMD) ----
    # each block gets PRIVATE tiles: its in-reach gaussians packed densely
    # into ceil(|S|/128) tiles (dup across blocks is fine; within a block
    # every in-reach gaussian appears exactly once)
    def _block_pass(c, idx, check_suffix):
        vy = p_int[idx, 1]
        gs = core_gsel[c]
        wlos, tb, strays = [], [], []
        for b in range(NBLK):
            vyb = vy[b * BLK:(b + 1) * BLK]
            desc = vyb[0] > vyb[-1]
            wlo = int(vyb[0]) - (KY_WIN - 1) if desc else int(vyb[0])
            wlos.append(wlo)
            inw = (vyb >= wlo) & (vyb < wlo + KY_WIN)
            ns = int((~inw).sum())
            strays.append(ns)
            if ns and check_suffix:
                assert inw[:BLK - ns].all(), "strays must be a suffix"
                assert b == NBLK - 1, "strays only in last slot"
            ylo, yhi = int(vyb.min()), int(vyb.max())
            S = gs[(m_int[gs, 1] + radii[gs] >= ylo)
                   & (m_int[gs, 1] - radii[gs] <= yhi)]
            lst = [S[k * P:(k + 1) * P]
                   for k in range(int(np.ceil(len(S) / P)))]
            tb.append(lst)
        return wlos, tb, strays

    # assign each core's blocks to SPMD slots so the padded per-slot pair
    # counts (elementwise max over cores) are minimal: stray block pinned to
    # the last slot, the rest sorted by pair count descending
    core_info = []
    npb = np.zeros(NBLK, dtype=np.int64)
    smax = 0
    for c in range(N_CORES):
        _, tb0, strays0 = _block_pass(c, core_idx[c], False)
        stray_bs = [b for b in range(NBLK) if strays0[b] > 0]
        assert len(stray_bs) <= 1
        sb = stray_bs[0] if stray_bs else \
            int(np.argmin([len(l) for l in tb0]))
        rest = sorted((b for b in range(NBLK) if b != sb),
                      key=lambda b: -len(tb0[b]))
        perm = rest + [sb]
        idx = np.concatenate([core_idx[c][b * BLK:(b + 1) * BLK]
                              for b in perm])
        core_idx[c] = idx
        wlos, tb, strays = _block_pass(c, idx, True)
        smax = max(smax, max(strays))
        for b in range(NBLK):
            npb[b] = max(npb[b], len(tb[b]))
        core_info.append((wlos, tb, strays))
    S = max(int(smax), 16)
    npair = int(npb.sum())
    # process the stray/addend slot (7) early, where its vector-engine mask
    # adds overlap pipeline slack; end on cheap 2-pair slots
    border = [0, 1, 7, 2, 3, 4, 5, 6]
    pair_block = []
    for b in border:
        pair_block += [b] * int(npb[b])
    add_pairs = [i for i, b in enumerate(pair_block) if b == NBLK - 1]

    # ---- per-core device arrays ----
    in_maps = []
    for c in range(N_CORES):
        idx = core_idx[c]
        gsel = core_gsel[c]
        wlos, tb, strays = core_info[c]
        vx = p_int[idx, 0]
        vy = p_int[idx, 1]
        vz = p_int[idx, 2]
        vx_lo = int(vx.min())
        pc = p[idx].astype(np.float64)

        feat = np.zeros((KT, NPC), dtype=FP16)
        stat = np.zeros((KT, npair * P), dtype=FP16)
        semt = np.zeros((P, npair * N_CLS), dtype=FP16)
        addn = np.zeros((P, len(add_pairs) * S), dtype=FP16)

        centers = np.stack([pc[b * BLK:(b + 1) * BLK].mean(axis=0)
                            for b in range(NBLK)])

        for b in range(NBLK):
            cols = slice(b * BLK, (b + 1) * BLK)
            dd = pc[cols] - centers[b]
            x, y, z = dd[:, 0], dd[:, 1], dd[:, 2]
            q = np.stack([x * x, y * y, z * z, x * y, y * z, x * z, x, y, z])
            qs = _split(q, 2)
            for f in range(KQ):
                for k, (i, _) in enumerate(NSPLIT):
                    feat[f * len(NSPLIT) + k, cols] = qs[i][f]
            feat[KQR:KQR + NBIAS, cols] = FP16(1)
            ar = np.arange(b * BLK, (b + 1) * BLK)
            feat[R_X + (vx[cols] - vx_lo), ar] = FP16(1)
            yr = vy[cols] - wlos[b]
            yrow = np.where((yr < 0) | (yr >= KY_WIN), KY_WIN,
                            np.clip(yr, 0, KY_WIN))
            feat[R_Y + yrow, ar] = FP16(1)
            feat[R_Z + vz[cols], ar] = FP16(1)

        pi = 0
        for b in border:
            lst = tb[b]
            for j in range(int(npb[b])):
                if j < len(lst):
                    gg = lst[j]
                    ng = len(gg)
                    mup = mu[gg].astype(np.float64) - centers[b]
                    mx, my, mz = mup[:, 0], mup[:, 1], mup[:, 2]
                    gxx, gyy, gzz = cxx[gg], cyy[gg], czz[gg]
                    gxy, gyz, gxz = cxy[gg], cyz[gg], cxz[gg]
                    hx = gxx * mx + gxy * my + gxz * mz
                    hy = gxy * mx + gyy * my + gyz * mz
                    hz = gxz * mx + gyz * my + gzz * mz
                    gq = np.stack([-0.5 * gxx, -0.5 * gyy, -0.5 * gzz,
                                   -gxy, -gyz, -gxz, hx, hy, hz])
                    gsp = _split(gq, 2)
                    sl = slice(pi * P, pi * P + ng)
                    for f in range(KQ):
                        for k, (_, jj) in enumerate(NSPLIT):
                            stat[f * len(NSPLIT) + k, sl] = gsp[jj][f]
                    quad = (gxx * mx * mx + gyy * my * my + gzz * mz * mz
                            + 2 * gxy * mx * my + 2 * gyz * my * mz
                            + 2 * gxz * mx * mz)
                    bias = -0.5 * quad + lnopa[gg]
                    for k, bsp in enumerate(_split(bias, NBIAS)):
                        stat[KQR + k, sl] = bsp
                    vv = np.arange(kx)[:, None] + vx_lo
                    out_x = np.abs(vv - m_int[gg, 0][None, :]) > radii[gg][None, :]
                    stat[R_X:R_X + kx, sl] = np.where(out_x, -MASKVAL, 0.0).astype(FP16)
                    vv = np.arange(KY_WIN)[:, None] + wlos[b]
                    out_y = np.abs(vv - m_int[gg, 1][None, :]) > radii[gg][None, :]
                    stat[R_Y:R_Y + KY_WIN, sl] = np.where(out_y, -MASKVAL, 0.0).astype(FP16)
                    # stray row R_Y + KY_WIN stays 0 (mask applied via addend)
                    vv = np.arange(KZ)[:, None]
                    out_z = np.abs(vv - m_int[gg, 2][None, :]) > radii[gg][None, :]
                    stat[R_Z:R_Z + KZ, sl] = np.where(out_z, -MASKVAL, 0.0).astype(FP16)
                    semt[:ng, pi * N_CLS:(pi + 1) * N_CLS] = sem[gg]
                    # addend: exact y mask for the stray suffix of last block
                    if b == NBLK - 1 and strays[b] > 0:
                        k = add_pairs.index(pi)
                        ns = strays[b]
                        svy = vy[b * BLK + BLK - ns:(b + 1) * BLK]
                        out_sy = (np.abs(m_int[gg, 1][:, None] - svy[None, :])
                                  > radii[gg][:, None])
                        addn[:ng, k * S + S - ns:(k + 1) * S] = \
                            np.where(out_sy, -MASKVAL, 0.0).astype(FP16)
                pi += 1

        in_maps.append({"feat": feat, "stat": stat, "semt": semt, "addn": addn})

    meta = dict(npair=npair, pair_block=pair_block, KT=KT,
                core_idx=core_idx, npb=npb, S=S, add_pairs=add_pairs)
    return in_maps, meta


def _build_nc(npair, pair_block, KT, S, add_pairs):
    import concourse.bass as bass  # noqa: F401
    import concourse.mybir as mybir
    import concourse.tile as tile
    from concourse import bacc

    f32 = mybir.dt.float32
    fp16 = mybir.dt.float16

    nc = bacc.Bacc("TRN2", target_bir_lowering=False, debug=False,
                   num_devices=N_CORES)
    feat_d = nc.dram_tensor("feat", [KT, NPC], fp16, kind="ExternalInput")
    stat_d = nc.dram_tensor("stat", [KT, npair * P], fp16, kind="ExternalInput")
    semt_d = nc.dram_tensor("semt", [P, npair * N_CLS], fp16, kind="ExternalInput")
    addn_d = nc.dram_tensor("addn", [P, len(add_pairs) * S], fp16,
                            kind="ExternalInput")
    out_d = nc.dram_tensor("out", [N_CLS, NPC], f32, kind="ExternalOutput")

    first = {}
    last = {}
    for i, b in enumerate(pair_block):
        first.setdefault(b, i)
        last[b] = i
    # groups of GRP pairs; shave one pair off the head group (releases the
    # first psum buffer sooner without adding a group) and keep a short tail
    grp_pairs = []
    i = 0
    while i < npair:
        n = GRP - 1 if i == 0 else (GRP if npair - i > 4 else 2)
        grp_pairs.append(list(range(i, min(i + n, npair))))
        i += n
    ngrp = len(grp_pairs)

    KH = KT // 2

    with tile.TileContext(nc) as tc:
        with (
            tc.tile_pool(name="resident", bufs=1) as res_pool,
            tc.tile_pool(name="wpool", bufs=4) as w_pool,
            tc.tile_pool(name="pwp", bufs=2, space="PSUM") as pw_pool,
            tc.tile_pool(name="lgp", bufs=1, space="PSUM") as lg_pool,
        ):
            feat_s = res_pool.tile([KT, NPC], fp16, name="feat_s")
            stat_s = res_pool.tile([KT, npair * P], fp16, name="stat_s")
            semt_s = res_pool.tile([P, npair * N_CLS], fp16, name="semt_s")
            addn_s = res_pool.tile([P, len(add_pairs) * S], fp16, name="addn_s")
            out_s = res_pool.tile([N_CLS, NPC], f32, name="out_s")

            def fdma(eng, rows, cols):
                eng.dma_start(out=feat_s[rows, cols], in_=feat_d[rows, cols])

            def sdma(eng, rows, plo, phi):
                eng.dma_start(out=stat_s[rows, plo * P:phi * P],
                              in_=stat_d[rows, plo * P:phi * P])

            k3 = KT // 3
            r1, r2, r3 = slice(0, k3), slice(k3, 2 * k3), slice(2 * k3, KT)
            allc = slice(0, NPC)

            # full-width rows only (4KB bursts), row-thirds across all three
            # queues so feat and the leading stat pairs land fastest
            rall = slice(0, KT)
            fdma(nc.sync, r1, allc)
            c1, c2, c3, c4 = (npair * k // 5 for k in (1, 2, 3, 4))
            sdma(nc.sync, rall, 0, c1)
            sdma(nc.sync, rall, c3, c4)

            fdma(nc.scalar, r2, allc)
            nc.scalar.dma_start(out=semt_s[:], in_=semt_d[:])
            sdma(nc.scalar, rall, c4, npair)

            fdma(nc.gpsimd, r3, allc)
            sdma(nc.gpsimd, rall, c1, c2)
            nc.gpsimd.dma_start(out=addn_s[:], in_=addn_d[:])
            sdma(nc.gpsimd, rall, c2, c3)

            lg = [lg_pool.tile([N_CLS, 2 * BLK], f32, name=f"lg{k}")
                  for k in range(NBLK // 2)]

            pw = [None] * ngrp
            w = [None] * ngrp

            def emit_splats(g):
                pw[g] = pw_pool.tile([P, len(grp_pairs[g]) * BLK], f32,
                                     name="pw")
                for s, i in enumerate(grp_pairs[g]):
                    b = pair_block[i]
                    nc.tensor.matmul(
                        out=pw[g][:, s * BLK:(s + 1) * BLK],
                        lhsT=stat_s[:, i * P:(i + 1) * P],
                        rhs=feat_s[:, b * BLK:(b + 1) * BLK],
                        start=True, stop=True)
                    if i in add_pairs:
                        k = add_pairs.index(i)
                        nc.vector.scalar_tensor_tensor(
                            out=pw[g][:, (s + 1) * BLK - S:(s + 1) * BLK],
                            in0=addn_s[:, k * S:(k + 1) * S],
                            scalar=1.0,
                            in1=pw[g][:, (s + 1) * BLK - S:(s + 1) * BLK],
                            op0=mybir.AluOpType.mult,
                            op1=mybir.AluOpType.add)

            def emit_exp(g):
                n = len(grp_pairs[g]) * BLK
                w[g] = w_pool.tile([P, n], fp16, name="w")
                nc.scalar.activation(w[g][:], pw[g][:],
                                     mybir.ActivationFunctionType.Exp)

            def emit_semts(g):
                for s, i in enumerate(grp_pairs[g]):
                    b = pair_block[i]
                    k, h = b // 2, b % 2
                    nc.tensor.matmul(
                        out=lg[k][:, h * BLK:(h + 1) * BLK],
                        lhsT=semt_s[:, i * N_CLS:(i + 1) * N_CLS],
                        rhs=w[g][:, s * BLK:(s + 1) * BLK],
                        start=(first[b] == i), stop=(last[b] == i))
                    if k == NBLK // 2 - 1 and last[b] == i:
                        # last bank: slot 7 completes mid-stream, so copy
                        # each half as its block finishes — only a 256-col
                        # copy remains on the critical tail
                        hs = slice((2 * k + h) * BLK, (2 * k + h + 1) * BLK)
                        nc.vector.tensor_copy(out_s[:, hs],
                                              lg[k][:, h * BLK:(h + 1) * BLK])
                    if max(last[2 * k], last[2 * k + 1]) == i:
                        sl = slice(k * 2 * BLK, (k + 1) * 2 * BLK)
                        if k < NBLK // 2 - 1:
                            nc.vector.tensor_copy(out_s[:, sl], lg[k][:])
                        dsl = slice(0, 4 * BLK) if k == 1 else sl
                        if k >= 1:
                            nc.sync.dma_start(out=out_d[:, dsl],
                                              in_=out_s[:, dsl])

            # semts trail the exps by two groups so they never wait on the
            # scalar engine; flush the last two groups at the end
            for g in range(ngrp):
                emit_splats(g)
                emit_exp(g)
                if g >= 2:
                    emit_semts(g - 2)
            emit_semts(ngrp - 2)
            emit_semts(ngrp - 1)

    nc.compile()
    return nc


def kernel(pts, means3D, opacities, semantics, scales, cov3D):
    global LAST_RESULTS
    from concourse.bass_utils import run_bass_kernel_spmd

    in_maps, meta = _prep(pts, means3D, opacities, semantics, scales, cov3D)
    nc = _build_nc(meta["npair"], meta["pair_block"], meta["KT"],
                   meta["S"], meta["add_pairs"])
    res = run_bass_kernel_spmd(nc, in_maps, core_ids=list(range(N_CORES)))
    LAST_RESULTS = res

    out = np.empty((N_PTS, N_CLS), dtype=np.float32)
    for c in range(N_CORES):
        out[meta["core_idx"][c]] = res.results[c]["out"].T
    return out


# revision 67
# speedup vs baseline: 1.0256x; 1.0053x over previous
import sys

sys.path.insert(0, "/opt/trn_rl_repo")

import numpy as np

# ---- problem constants (hardcoded from the nn_LocalAggregator spec) ----
PC_MIN = np.array([-40.0, -40.0, -1.0], dtype=np.float32)
GRID = np.float32(0.4)
SCALE_MULT = np.float32(3.0)
N_PTS, N_GAUSS, N_CLS = 16384, 4096, 18
N_CORES = 8
NPC = N_PTS // N_CORES          # 2048 points per core
BLK = 256                       # point block (matmul free dim)
NBLK = NPC // BLK               # 8
P = 128                         # partitions / gaussians per tile
KQ = 9                          # quadratic+linear monomial features
NSPLIT = [(0, 0), (0, 1), (1, 0)]  # fp16 2-level split combos
KQR = KQ * len(NSPLIT)          # 27 quad rows after splitting
NBIAS = 2                       # bias folded in as 2 fp16 rows (feat = 1)
KQB = KQR + NBIAS               # 29 dense rows
KY_WIN = 17                     # in-window y voxels
KYR = KY_WIN + 1                # + 1 "stray" row (no y penalty, addend fixes)
KZ = 16                         # z voxel range
GRP = 4                         # pairs per merged exp (one 2-bank psum tile)

FP16 = np.float16
MASKVAL = np.float64(240.0)     # mask penalty per violated axis (fp16-exact)

# module global for test harness introspection (exec time etc.)
LAST_RESULTS = None


def _split(x, n):
    """float64 array -> n fp16 levels whose sum approximates x."""
    out = []
    r = np.asarray(x, dtype=np.float64)
    for _ in range(n):
        a = r.astype(FP16)
        out.append(a)
        r = r - a.astype(np.float64)
    return out


def _prep(pts, means3D, opacities, semantics, scales, cov3D):
    """Host-side prep: sharding, features, coefficient tables."""
    p = np.asarray(pts[0], dtype=np.float32)          # [N,3]
    mu = np.asarray(means3D[0], dtype=np.float32)     # [M,3]
    opa = np.asarray(opacities[0], dtype=np.float32)  # [M]
    sem = np.asarray(semantics[0], dtype=np.float32)  # [M,C]
    sc = np.asarray(scales[0], dtype=np.float32)      # [M,3]
    cov = np.asarray(cov3D[0], dtype=np.float32)      # [M,3,3]

    # integer voxel coords / radii -- fp32 ops exactly as the reference
    p_int = ((p - PC_MIN) / GRID).astype(np.int32)
    m_int = ((mu - PC_MIN) / GRID).astype(np.int32)
    radii = np.ceil(sc.max(axis=-1) * SCALE_MULT / GRID).astype(np.int32)

    cxx = cov[:, 0, 0].astype(np.float64)
    cyy = cov[:, 1, 1].astype(np.float64)
    czz = cov[:, 2, 2].astype(np.float64)
    cxy = cov[:, 0, 1].astype(np.float64)
    cyz = cov[:, 1, 2].astype(np.float64)
    cxz = cov[:, 0, 2].astype(np.float64)
    with np.errstate(divide="ignore"):
        lnopa = np.maximum(np.log(opa.astype(np.float64)), -20000.0)

    # ---- shard points: equal x-chunks, y-sorted inside each core; pick the
    # sort direction so the wide-span (stray) block is always the LAST one ----
    order_x = np.argsort(p[:, 0], kind="stable")
    core_idx = []
    for c in range(N_CORES):
        idx = order_x[c * NPC:(c + 1) * NPC]
        idx = idx[np.argsort(p[idx, 1], kind="stable")]
        vy = p_int[idx, 1]
        if vy[:BLK].max() - vy[:BLK].min() > vy[-BLK:].max() - vy[-BLK:].min():
            idx = idx[::-1]
        core_idx.append(idx)

    # ---- per-core gaussian subsets (x-reach cull), y-sorted ----
    core_gsel = []
    kx = 0
    for c in range(N_CORES):
        vx = p_int[core_idx[c], 0]
        kx = max(kx, int(vx.max() - vx.min()) + 1)
        m = (m_int[:, 0] >= vx.min() - radii) & (m_int[:, 0] <= vx.max() + radii)
        gsel = np.nonzero(m)[0]
        gsel = gsel[np.argsort(m_int[gsel, 1], kind="stable")]
        core_gsel.append(gsel)

    KT = KQB + kx + KYR + KZ            # total contraction rows
    assert KT <= 128, KT
    R_X = KQB
    R_Y = KQB + kx
    R_Z = KQB + kx + KYR

    # ---- per-core block windows + pair lists; pad across cores (SPMD) ----
    # each block gets PRIVATE tiles: its in-reach gaussians packed densely
    # into ceil(|S|/128) tiles (dup across blocks is fine; within a block
    # every in-reach gaussian appears exactly once)
    def _block_pass(c, idx, check_suffix):
        vy = p_int[idx, 1]
        gs = core_gsel[c]
        wlos, tb, strays = [], [], []
        for b in range(NBLK):
            vyb = vy[b * BLK:(b + 1) * BLK]
            desc = vyb[0] > vyb[-1]
            wlo = int(vyb[0]) - (KY_WIN - 1) if desc else int(vyb[0])
            wlos.append(wlo)
            inw = (vyb >= wlo) & (vyb < wlo + KY_WIN)
            ns = int((~inw).sum())
            strays.append(ns)
            if ns and check_suffix:
                assert inw[:BLK - ns].all(), "strays must be a suffix"
                assert b == NBLK - 1, "strays only in last slot"
            ylo, yhi = int(vyb.min()), int(vyb.max())
            S = gs[(m_int[gs, 1] + radii[gs] >= ylo)
                   & (m_int[gs, 1] - radii[gs] <= yhi)]
            lst = [S[k * P:(k + 1) * P]
                   for k in range(int(np.ceil(len(S) / P)))]
            tb.append(lst)
        return wlos, tb, strays

    # assign each core's blocks to SPMD slots so the padded per-slot pair
    # counts (elementwise max over cores) are minimal: stray block pinned to
    # the last slot, the rest sorted by pair count descending
    core_info = []
    npb = np.zeros(NBLK, dtype=np.int64)
    smax = 0
    for c in range(N_CORES):
        _, tb0, strays0 = _block_pass(c, core_idx[c], False)
        stray_bs = [b for b in range(NBLK) if strays0[b] > 0]
        assert len(stray_bs) <= 1
        sb = stray_bs[0] if stray_bs else \
            int(np.argmin([len(l) for l in tb0]))
        rest = sorted((b for b in range(NBLK) if b != sb),
                      key=lambda b: -len(tb0[b]))
        perm = rest + [sb]
        idx = np.concatenate([core_idx[c][b * BLK:(b + 1) * BLK]
                              for b in perm])
        core_idx[c] = idx
        wlos, tb, strays = _block_pass(c, idx, True)
        smax = max(smax, max(strays))
        for b in range(NBLK):
            npb[b] = max(npb[b], len(tb[b]))
        core_info.append((wlos, tb, strays))
    S = max(int(smax), 16)
    npair = int(npb.sum())
    # process the stray/addend slot (7) early, where its vector-engine mask
    # adds overlap pipeline slack; end on cheap 2-pair slots
    border = [0, 1, 7, 2, 3, 4, 5, 6]
    pair_block = []
    for b in border:
        pair_block += [b] * int(npb[b])
    add_pairs = [i for i, b in enumerate(pair_block) if b == NBLK - 1]

    # ---- per-core device arrays ----
    in_maps = []
    for c in range(N_CORES):
        idx = core_idx[c]
        gsel = core_gsel[c]
        wlos, tb, strays = core_info[c]
        vx = p_int[idx, 0]
        vy = p_int[idx, 1]
        vz = p_int[idx, 2]
        vx_lo = int(vx.min())
        pc = p[idx].astype(np.float64)

        feat = np.zeros((KT, NPC), dtype=FP16)
        stat = np.zeros((KT, npair * P), dtype=FP16)
        semt = np.zeros((P, npair * N_CLS), dtype=FP16)
        addn = np.zeros((P, len(add_pairs) * S), dtype=FP16)

        centers = np.stack([pc[b * BLK:(b + 1) * BLK].mean(axis=0)
                            for b in range(NBLK)])

        for b in range(NBLK):
            cols = slice(b * BLK, (b + 1) * BLK)
            dd = pc[cols] - centers[b]
            x, y, z = dd[:, 0], dd[:, 1], dd[:, 2]
            q = np.stack([x * x, y * y, z * z, x * y, y * z, x * z, x, y, z])
            qs = _split(q, 2)
            for f in range(KQ):
                for k, (i, _) in enumerate(NSPLIT):
                    feat[f * len(NSPLIT) + k, cols] = qs[i][f]
            feat[KQR:KQR + NBIAS, cols] = FP16(1)
            ar = np.arange(b * BLK, (b + 1) * BLK)
            feat[R_X + (vx[cols] - vx_lo), ar] = FP16(1)
            yr = vy[cols] - wlos[b]
            yrow = np.where((yr < 0) | (yr >= KY_WIN), KY_WIN,
                            np.clip(yr, 0, KY_WIN))
            feat[R_Y + yrow, ar] = FP16(1)
            feat[R_Z + vz[cols], ar] = FP16(1)

        pi = 0
        for b in border:
            lst = tb[b]
            for j in range(int(npb[b])):
                if j < len(lst):
                    gg = lst[j]
                    ng = len(gg)
                    mup = mu[gg].astype(np.float64) - centers[b]
                    mx, my, mz = mup[:, 0], mup[:, 1], mup[:, 2]
                    gxx, gyy, gzz = cxx[gg], cyy[gg], czz[gg]
                    gxy, gyz, gxz = cxy[gg], cyz[gg], cxz[gg]
                    hx = gxx * mx + gxy * my + gxz * mz
                    hy = gxy * mx + gyy * my + gyz * mz
                    hz = gxz * mx + gyz * my + gzz * mz
                    gq = np.stack([-0.5 * gxx, -0.5 * gyy, -0.5 * gzz,
                                   -gxy, -gyz, -gxz, hx, hy, hz])
                    gsp = _split(gq, 2)
                    sl = slice(pi * P, pi * P + ng)
                    for f in range(KQ):
                        for k, (_, jj) in enumerate(NSPLIT):
                            stat[f * len(NSPLIT) + k, sl] = gsp[jj][f]
                    quad = (gxx * mx * mx + gyy * my * my + gzz * mz * mz
                            + 2 * gxy * mx * my + 2 * gyz * my * mz
                            + 2 * gxz * mx * mz)
                    bias = -0.5 * quad + lnopa[gg]
                    for k, bsp in enumerate(_split(bias, NBIAS)):
                        stat[KQR + k, sl] = bsp
                    vv = np.arange(kx)[:, None] + vx_lo
                    out_x = np.abs(vv - m_int[gg, 0][None, :]) > radii[gg][None, :]
                    stat[R_X:R_X + kx, sl] = np.where(out_x, -MASKVAL, 0.0).astype(FP16)
                    vv = np.arange(KY_WIN)[:, None] + wlos[b]
                    out_y = np.abs(vv - m_int[gg, 1][None, :]) > radii[gg][None, :]
                    stat[R_Y:R_Y + KY_WIN, sl] = np.where(out_y, -MASKVAL, 0.0).astype(FP16)
                    # stray row R_Y + KY_WIN stays 0 (mask applied via addend)
                    vv = np.arange(KZ)[:, None]
                    out_z = np.abs(vv - m_int[gg, 2][None, :]) > radii[gg][None, :]
                    stat[R_Z:R_Z + KZ, sl] = np.where(out_z, -MASKVAL, 0.0).astype(FP16)
                    semt[:ng, pi * N_CLS:(pi + 1) * N_CLS] = sem[gg]
                    # addend: exact y mask for the stray suffix of last block
                    if b == NBLK - 1 and strays[b] > 0:
                        k = add_pairs.index(pi)
                        ns = strays[b]
                        svy = vy[b * BLK + BLK - ns:(b + 1) * BLK]
                        out_sy = (np.abs(m_int[gg, 1][:, None] - svy[None, :])
                                  > radii[gg][:, None])
                        addn[:ng, k * S + S - ns:(k + 1) * S] = \
                            np.where(out_sy, -MASKVAL, 0.0).astype(FP16)
                pi += 1

        in_maps.append({"feat": feat, "stat": stat, "semt": semt, "addn": addn})

    meta = dict(npair=npair, pair_block=pair_block, KT=KT,
                core_idx=core_idx, npb=npb, S=S, add_pairs=add_pairs)
    return in_maps, meta


def _build_nc(npair, pair_block, KT, S, add_pairs):
    import concourse.bass as bass  # noqa: F401
    import concourse.mybir as mybir
    import concourse.tile as tile
    from concourse import bacc

    f32 = mybir.dt.float32
    fp16 = mybir.dt.float16

    nc = bacc.Bacc("TRN2", target_bir_lowering=False, debug=False,
                   num_devices=N_CORES)
    feat_d = nc.dram_tensor("feat", [KT, NPC], fp16, kind="ExternalInput")
    stat_d = nc.dram_tensor("stat", [KT, npair * P], fp16, kind="ExternalInput")
    semt_d = nc.dram_tensor("semt", [P, npair * N_CLS], fp16, kind="ExternalInput")
    addn_d = nc.dram_tensor("addn", [P, len(add_pairs) * S], fp16,
                            kind="ExternalInput")
    out_d = nc.dram_tensor("out", [N_CLS, NPC], f32, kind="ExternalOutput")

    first = {}
    last = {}
    for i, b in enumerate(pair_block):
        first.setdefault(b, i)
        last[b] = i
    # groups of GRP pairs; shave one pair off the head group (releases the
    # first psum buffer sooner without adding a group) and keep a short tail
    grp_pairs = []
    i = 0
    while i < npair:
        n = GRP - 1 if i == 0 else (GRP if npair - i > 4 else 2)
        grp_pairs.append(list(range(i, min(i + n, npair))))
        i += n
    ngrp = len(grp_pairs)

    KH = KT // 2

    with tile.TileContext(nc) as tc:
        with (
            tc.tile_pool(name="resident", bufs=1) as res_pool,
            tc.tile_pool(name="wpool", bufs=4) as w_pool,
            tc.tile_pool(name="pwp", bufs=2, space="PSUM") as pw_pool,
            tc.tile_pool(name="lgp", bufs=1, space="PSUM") as lg_pool,
        ):
            feat_s = res_pool.tile([KT, NPC], fp16, name="feat_s")
            stat_s = res_pool.tile([KT, npair * P], fp16, name="stat_s")
            semt_s = res_pool.tile([P, npair * N_CLS], fp16, name="semt_s")
            addn_s = res_pool.tile([P, len(add_pairs) * S], fp16, name="addn_s")
            out_s = res_pool.tile([N_CLS, NPC], f32, name="out_s")

            def fdma(eng, rows, cols):
                eng.dma_start(out=feat_s[rows, cols], in_=feat_d[rows, cols])

            def sdma(eng, rows, plo, phi):
                eng.dma_start(out=stat_s[rows, plo * P:phi * P],
                              in_=stat_d[rows, plo * P:phi * P])

            k3 = KT // 3
            r1, r2, r3 = slice(0, k3), slice(k3, 2 * k3), slice(2 * k3, KT)
            allc = slice(0, NPC)

            # full-width rows only (4KB bursts), row-thirds across all three
            # queues so feat and the leading stat pairs land fastest
            rall = slice(0, KT)
            fdma(nc.sync, r1, allc)
            c1, c2, c3, c4 = (npair * k // 5 for k in (1, 2, 3, 4))
            sdma(nc.sync, rall, 0, c1)
            sdma(nc.sync, rall, c3, c4)

            fdma(nc.scalar, r2, allc)
            nc.scalar.dma_start(out=semt_s[:], in_=semt_d[:])
            sdma(nc.scalar, rall, c4, npair)

            fdma(nc.gpsimd, r3, allc)
            sdma(nc.gpsimd, rall, c1, c2)
            sdma(nc.gpsimd, rall, c2, c3)
            nc.gpsimd.dma_start(out=addn_s[:], in_=addn_d[:])

            lg = [lg_pool.tile([N_CLS, 2 * BLK], f32, name=f"lg{k}")
                  for k in range(NBLK // 2)]

            pw = [None] * ngrp
            w = [None] * ngrp

            def emit_splats(g):
                pw[g] = pw_pool.tile([P, len(grp_pairs[g]) * BLK], f32,
                                     name="pw")
                for s, i in enumerate(grp_pairs[g]):
                    b = pair_block[i]
                    nc.tensor.matmul(
                        out=pw[g][:, s * BLK:(s + 1) * BLK],
                        lhsT=stat_s[:, i * P:(i + 1) * P],
                        rhs=feat_s[:, b * BLK:(b + 1) * BLK],
                        start=True, stop=True)
                    if i in add_pairs:
                        k = add_pairs.index(i)
                        nc.vector.scalar_tensor_tensor(
                            out=pw[g][:, (s + 1) * BLK - S:(s + 1) * BLK],
                            in0=addn_s[:, k * S:(k + 1) * S],
                            scalar=1.0,
                            in1=pw[g][:, (s + 1) * BLK - S:(s + 1) * BLK],
                            op0=mybir.AluOpType.mult,
                            op1=mybir.AluOpType.add)

            def emit_exp(g):
                n = len(grp_pairs[g]) * BLK
                w[g] = w_pool.tile([P, n], fp16, name="w")
                nc.scalar.activation(w[g][:], pw[g][:],
                                     mybir.ActivationFunctionType.Exp)

            def emit_semts(g):
                for s, i in enumerate(grp_pairs[g]):
                    b = pair_block[i]
                    k, h = b // 2, b % 2
                    nc.tensor.matmul(
                        out=lg[k][:, h * BLK:(h + 1) * BLK],
                        lhsT=semt_s[:, i * N_CLS:(i + 1) * N_CLS],
                        rhs=w[g][:, s * BLK:(s + 1) * BLK],
                        start=(first[b] == i), stop=(last[b] == i))
                    if k == NBLK // 2 - 1 and last[b] == i:
                        # last bank: slot 7 completes mid-stream, so copy
                        # each half as its block finishes — only a 256-col
                        # copy remains on the critical tail
                        hs = slice((2 * k + h) * BLK, (2 * k + h + 1) * BLK)
                        nc.vector.tensor_copy(out_s[:, hs],
                                              lg[k][:, h * BLK:(h + 1) * BLK])
                    if max(last[2 * k], last[2 * k + 1]) == i:
                        sl = slice(k * 2 * BLK, (k + 1) * 2 * BLK)
                        if k < NBLK // 2 - 1:
                            nc.vector.tensor_copy(out_s[:, sl], lg[k][:])
                        dsl = slice(0, 4 * BLK) if k == 1 else sl
                        if k >= 1:
                            nc.sync.dma_start(out=out_d[:, dsl],
                                              in_=out_s[:, dsl])

            # semts trail the exps by two groups so they never wait on the
            # scalar engine; flush the last two groups at the end
            for g in range(ngrp):
                emit_splats(g)
                emit_exp(g)
                if g >= 2:
                    emit_semts(g - 2)
            emit_semts(ngrp - 2)
            emit_semts(ngrp - 1)

    nc.compile()
    return nc


def kernel(pts, means3D, opacities, semantics, scales, cov3D):
    global LAST_RESULTS
    from concourse.bass_utils import run_bass_kernel_spmd

    in_maps, meta = _prep(pts, means3D, opacities, semantics, scales, cov3D)
    nc = _build_nc(meta["npair"], meta["pair_block"], meta["KT"],
                   meta["S"], meta["add_pairs"])
    res = run_bass_kernel_spmd(nc, in_maps, core_ids=list(range(N_CORES)))
    LAST_RESULTS = res

    out = np.empty((N_PTS, N_CLS), dtype=np.float32)
    for c in range(N_CORES):
        out[meta["core_idx"][c]] = res.results[c]["out"].T
    return out
